# revision 41
# baseline (speedup 1.0000x reference)
"""Trainium2 Bass kernel for the multi-task ActorNetwork (moe_routing).

Architecture (reference): per-sample expert routing over G=8 tasks:
    h1 = relu(x @ W1[idx] + b1[idx])     x:[B,376]  W1:[8,376,400]
    hf = relu(h1 @ W2 + b2)              W2:[400,300]
    a  = tanh(hf @ W3[idx] + b3[idx])    W3:[8,300,17]

Strategy: idx is sorted, and G == n_cores == 8, so we route on the HOST:
core g receives exactly the contiguous rows with idx == g (zero-padded to a
common BM), plus only ITS expert weights. Each core then runs a dense 3-layer
MLP -- no device-side routing, no collectives, and none of the 8x dense
compute the reference does.

Numerics: fp16 operands with fp32 PSUM accumulation (fp16 matmul streams at
1 cycle/row on the PE vs 4 for fp32; measured end-to-end max-abs error vs the
fp32 reference ~5e-3 on unit-scale outputs).

Layout: all matmuls keep the contraction dim on SBUF partitions:
    L1: h1T[h1, b] = relu(W1[d,h1].T @ xT[d,b] + b1)   (xT pre-transposed on host)
    L2: hfT[h2, b] = relu(W2[h1,h2].T @ h1T[h1,b] + b2)
    L3: aT[a, b]   = tanh(W3[h2,a].T @ hfT[h2,b] + b3) (host transposes back)
Biases ride the per-partition bias operand of the PSUM-eviction op (ACT
activation / DVE tensor_scalar), so each layer is matmul + one eviction op.

Engine split: PE matmuls; ACT does L1-relu + L3-tanh; DVE does L2-relu;
x-chunks stream on the SP HWDGE ring, weights on the Pool SWDGE ring, outputs
on the ACT HWDGE ring. A few dummy matmuls at t=0 warm the PE p-state while
the first DMAs land.
"""

import sys

if "/opt/trn_rl_repo" not in sys.path:
    sys.path.insert(0, "/opt/trn_rl_repo")

from contextlib import ExitStack

import numpy as np

import concourse.bass as bass
import concourse.mybir as mybir
from concourse.bass_utils import run_bass_kernel_spmd
from concourse.tile import TileContext

D, G, H1, H2, A = 376, 8, 400, 300, 17
P = 128
NCORES = 8
F16 = mybir.dt.float16
F32 = mybir.dt.float32


def _chunks(total, step):
    return [(o, min(step, total - o)) for o in range(0, total, step)]


K1 = _chunks(D, P)  # contraction tiles, layer 1: (128,128,120)
M1 = _chunks(H1, P)  # output-row tiles,  layer 1: (128,128,128,16)
K2 = M1  # contraction tiles, layer 2 == layer-1 output tiling
M2 = _chunks(H2, P)  # output-row tiles,  layer 2: (128,128,44)
K3 = M2  # contraction tiles, layer 3 == layer-2 output tiling

# packed per-partition bias columns: [128, 8] = b1 x4 | b2 x3 | b3 x1
BIAS_COLS = len(M1) + len(M2) + 1

_nc_cache = {}
last_run = None  # BassKernelResults of the most recent launch (for profiling)
_last_in_maps = None  # per-core input dicts of the most recent launch

_nop_counter = [0]


def _legalize_wait_counts(nc):
    """This container's walrus encodes at most ONE sync-wait per instruction
    (DMA pseudo-instructions especially). Tile freely emits several. Sequencers
    are in-order, so hoisting the surplus waits onto same-engine NoOps placed
    immediately before the instruction is semantics-preserving."""
    for fn in nc.m.functions:
        for bb in fn.blocks:
            insts = list(bb.instructions)
            out = []
            changed = False
            for inst in insts:
                si = inst.sync_info
                waits = list(si.on_wait) if si is not None and si.on_wait else []
                if len(waits) > 1:
                    changed = True
                    for w in waits[:-1]:
                        _nop_counter[0] += 1
                        nop = mybir.InstNoOp(
                            name=f"waitsplit_nop_{_nop_counter[0]}",
                            engine=inst.engine,
                            ins=[],
                            outs=[],
                            sync_info=mybir.SyncInfo(on_wait=[w], on_update=[]),
                        )
                        out.append(nop)
                    si.on_wait = waits[-1:]
                out.append(inst)
            if changed:
                bb.instructions = out
    return nc


def _build(BM, legalize=True, reps=1):
    """Bass program for one core: dense 3-layer MLP over BM rows.

    reps>1 wraps the body in a hardware For_i loop (benchmarking only)."""
    bchunks = _chunks(BM, 512)

    nc = bass.Bass()
    xT = nc.declare_dram_parameter("xT", [D, BM], F16, isOutput=False)
    w1 = nc.declare_dram_parameter("w1", [D, H1], F16, isOutput=False)
    w2 = nc.declare_dram_parameter("w2", [H1, H2], F16, isOutput=False)
    w3 = nc.declare_dram_parameter("w3", [H2, A], F16, isOutput=False)
    bias = nc.declare_dram_parameter("bias", [P, BIAS_COLS], F32, isOutput=False)
    out = nc.declare_dram_parameter("out", [A, BM], F32, isOutput=True)

    Relu = mybir.ActivationFunctionType.Relu
    Tanh = mybir.ActivationFunctionType.Tanh
    Add = mybir.AluOpType.add
    Max = mybir.AluOpType.max

    with TileContext(nc) as tc, ExitStack() as ctx:
        wpool = ctx.enter_context(tc.tile_pool(name="w", bufs=1))
        xpool = ctx.enter_context(tc.tile_pool(name="x", bufs=3))
        h1pool = ctx.enter_context(tc.tile_pool(name="h1", bufs=3))
        hfpool = ctx.enter_context(tc.tile_pool(name="hf", bufs=3))
        opool = ctx.enter_context(tc.tile_pool(name="o", bufs=3))
        ps1 = ctx.enter_context(tc.tile_pool(name="ps1", bufs=4, space="PSUM"))
        ps2 = ctx.enter_context(tc.tile_pool(name="ps2", bufs=3, space="PSUM"))
        ps3 = ctx.enter_context(tc.tile_pool(name="ps3", bufs=1, space="PSUM"))

        def load_weights(param, kchunks, ncols, name, eng):
            # weights ride the idle Pool/ACT rings so they don't serialize
            # behind the x-chunk stream on the SP/HWDGE ring
            tiles = []
            for i, (k0, ks) in enumerate(kchunks):
                t = wpool.tile([ks, ncols], F16, tag=f"{name}_{i}")
                eng.dma_start(out=t[:, :], in_=param[k0 : k0 + ks, :])
                tiles.append(t)
            return tiles

        w1_t = load_weights(w1, K1, H1, "w1", nc.gpsimd)
        bias_t = wpool.tile([P, BIAS_COLS], F32, tag="bias")
        nc.gpsimd.dma_start(out=bias_t[:, :], in_=bias[:, :])
        w2_t = load_weights(w2, K2, H2, "w2", nc.scalar)
        w3_t = load_weights(w3, K3, A, "w3", nc.scalar)

        def b1_ap(mi, ms):
            return bias_t[:ms, mi : mi + 1]

        def b2_ap(mi, ms):
            return bias_t[:ms, len(M1) + mi : len(M1) + mi + 1]

        def b3_ap():
            return bias_t[:A, BIAS_COLS - 1 : BIAS_COLS]

        # p-state warmup: ~3us of dummy matmuls on zeroed SBUF while the
        # first DMAs are in flight, so the real matmuls run at 2.4 GHz
        warm = wpool.tile([P, 512], F16, tag="warm")
        nc.vector.memset(warm[:, :], 0.0)
        for _ in range(10):
            pw = ps3.tile([P, 512], F32, tag="ps3")
            nc.tensor.matmul(
                pw[:, :512], warm[:, :P], warm[:, :512], start=True, stop=True
            )

        def emit_l1(b0, nb):
            # stream this chunk of pre-transposed activations in
            x_t = []
            for ki, (k0, ks) in enumerate(K1):
                t = xpool.tile([ks, nb], F16, tag=f"x{ki}")
                nc.sync.dma_start(out=t[:, :nb], in_=xT[k0 : k0 + ks, b0 : b0 + nb])
                x_t.append(t)

            # ---- layer 1: h1T[h1, b] = relu(W1.T @ xT + b1) ----
            h1_t = []
            for mi, (m0, ms) in enumerate(M1):
                pt = ps1.tile([P, 512], F32, tag="ps1")
                for ki in range(len(K1)):
                    nc.tensor.matmul(
                        pt[:ms, :nb],
                        w1_t[ki][:, m0 : m0 + ms],
                        x_t[ki][:, :nb],
                        start=(ki == 0),
                        stop=(ki == len(K1) - 1),
                    )
                ht = h1pool.tile([ms, nb], F16, tag=f"h1_{mi}")
                if mi == len(M1) - 1:
                    # the 16-row remainder costs a full tile-pass on whichever
                    # engine runs it; DVE has the most slack
                    nc.vector.tensor_scalar(
                        ht[:ms, :nb], pt[:ms, :nb], b1_ap(mi, ms), 0.0, op0=Add, op1=Max
                    )
                else:
                    nc.scalar.activation(
                        ht[:ms, :nb], pt[:ms, :nb], Relu, bias=b1_ap(mi, ms)
                    )
                h1_t.append(ht)
            return h1_t

        def emit_l2(h1_t, nb):
            # ---- layer 2: hfT[h2, b] = relu(W2.T @ h1T + b2), relu on DVE ----
            hf_t = []
            for mi, (m0, ms) in enumerate(M2):
                pt = ps2.tile([P, 512], F32, tag="ps2")
                for ki in range(len(K2)):
                    nc.tensor.matmul(
                        pt[:ms, :nb],
                        w2_t[ki][:, m0 : m0 + ms],
                        h1_t[ki][:, :nb],
                        start=(ki == 0),
                        stop=(ki == len(K2) - 1),
                    )
                ht = hfpool.tile([ms, nb], F16, tag=f"hf_{mi}")
                nc.vector.tensor_scalar(
                    ht[:ms, :nb], pt[:ms, :nb], b2_ap(mi, ms), 0.0, op0=Add, op1=Max
                )
                hf_t.append(ht)
            return hf_t

        def emit_l3(hf_t, b0, nb):
            # ---- layer 3: aT[a, b] = tanh(W3.T @ hfT + b3) ----
            pt = ps3.tile([P, 512], F32, tag="ps3")
            for ki in range(len(K3)):
                nc.tensor.matmul(
                    pt[:A, :nb],
                    w3_t[ki][:, :A],
                    hf_t[ki][:, :nb],
                    start=(ki == 0),
                    stop=(ki == len(K3) - 1),
                )
            ot = opool.tile([A, nb], F32, tag="o")
            nc.scalar.activation(ot[:A, :nb], pt[:A, :nb], Tanh, bias=b3_ap())
            # out DMA on the ACT HWDGE ring: it trails tanh on the same
            # sequencer, so its wait never blocks the SP ring's x-prefetches
            nc.scalar.dma_start(out=out[:, b0 : b0 + nb], in_=ot[:A, :nb])

        def emit_all():
            # software-pipelined emission: L3 of chunk c-1 sits between L1(c)
            # and L2(c) in the PE stream, so the PE never waits on a relu that
            # was issued immediately before
            pending = None
            for b0, nb in bchunks:
                h1_t = emit_l1(b0, nb)
                if pending is not None:
                    emit_l3(*pending)
                hf_t = emit_l2(h1_t, nb)
                pending = (hf_t, b0, nb)
            emit_l3(*pending)

        if reps > 1:
            with tc.For_i(0, reps, 1):
                emit_all()
        else:
            emit_all()
    return _legalize_wait_counts(nc) if legalize else nc


def _get_nc(BM):
    if BM not in _nc_cache:
        _nc_cache[BM] = _build(BM)
    return _nc_cache[BM]


def kernel(state, idx, W1, b1, W2, b2, W3, b3):
    global last_run
    state = np.asarray(state, dtype=np.float32)
    idx = np.asarray(idx)
    W1 = np.asarray(W1, dtype=np.float32)
    b1 = np.asarray(b1, dtype=np.float32)
    W2 = np.asarray(W2, dtype=np.float32)
    b2 = np.asarray(b2, dtype=np.float32)
    W3 = np.asarray(W3, dtype=np.float32)
    b3 = np.asarray(b3, dtype=np.float32)
    B = state.shape[0]

    # Host-side routing: idx is sorted in the reference workload; fall back to
    # a stable argsort if not, so grouping stays correct for any input.
    idx_i = idx.astype(np.int64)
    perm = None
    if np.any(np.diff(idx_i) < 0):
        perm = np.argsort(idx_i, kind="stable")
        idx_i = idx_i[perm]
        state = state[perm]
    assert idx_i.min() >= 0 and idx_i.max() < G, "idx out of range [0, G)"
    counts = np.bincount(idx_i, minlength=G)[:G]
    offs = np.concatenate([[0], np.cumsum(counts)])

    BM = max(512, int(-(-counts.max() // P) * P))  # round up to 128 rows
    nc = _get_nc(BM)

    w2f = W2.astype(np.float16)

    def pack_bias(b1g, b3g):
        pk = np.zeros((P, BIAS_COLS), np.float32)
        for mi, (m0, ms) in enumerate(M1):
            pk[:ms, mi] = b1g[m0 : m0 + ms]
        for mi, (m0, ms) in enumerate(M2):
            pk[:ms, len(M1) + mi] = b2[m0 : m0 + ms]
        pk[:A, BIAS_COLS - 1] = b3g
        return pk

    in_maps = []
    for g in range(G):
        seg = state[offs[g] : offs[g + 1]]
        xg = np.zeros((D, BM), np.float16)
        xg[:, : seg.shape[0]] = seg.T.astype(np.float16)
        in_maps.append(
            {
                "xT": xg,
                "w1": W1[g].astype(np.float16),
                "w2": w2f,
                "w3": W3[g].astype(np.float16),
                "bias": pack_bias(b1[g], b3[g]),
            }
        )

    globals()["_last_in_maps"] = in_maps
    try:
        last_run = run_bass_kernel_spmd(nc, in_maps, list(range(NCORES)))
    except ModuleNotFoundError:
        # BASS_TRACE set in an env without the axon NTFF hook: retry untraced
        import os

        os.environ["BASS_NEVER_TRACE"] = "1"
        last_run = run_bass_kernel_spmd(nc, in_maps, list(range(NCORES)))

    out = np.empty((B, A), np.float32)
    for g in range(G):
        og = np.asarray(last_run.results[g]["out"])  # [A, BM]
        out[offs[g] : offs[g + 1]] = og.T[: counts[g]]
    if perm is not None:
        inv = np.empty_like(perm)
        inv[perm] = np.arange(B)
        out = out[inv]
    return out
